# revision 61
# baseline (speedup 1.0000x reference)
"""Trainium2 Bass kernel for the ActorCritic ragged-sequence problem.

Strategy
--------
Data-parallel over batch B=64 across 8 NeuronCores, with *valid-row
packing*: the reference masks every pair position t >= length-1, so only
sum(lengths) columns of work exist (~56% of B*S for uniform lengths).
The host assigns batch rows to cores to balance sum(lengths) (swap-based
local search), packs each core's rows' first `len` states contiguously
into columns, and the kernel processes only ceil(max_core_sum/512)
strips of 512 rows instead of the dense 16.  The +1 shift of the pair's
"second" element stays a one-element column offset inside each segment;
each segment's last column becomes a garbage "pair" that the softmax
masks out, exactly like the reference's -inf positions.

Per core the pair-MLP runs as weight-stationary fp8 DoubleRow matmuls
(K=256/instruction) in groups of <=5 row-slices per PSUM generation.
All strips of a group accumulate their w2p score dot into ONE M=8*qsg
PSUM tile (strip j's replicated-w2p stationary is nonzero only at
columns [8j, 8j+8), so accumulation adds zeros elsewhere); a single ACT
copy descales the whole group into an SBUF score grid whose partition
8*rs+b is strip rs "owned" by batch row b — no per-strip DMAs or
semaphore latency on the tail.  Host-built masks (0/-1e30 segment masks
and a position one-hot) turn masked log-softmax + entropy per batch row
into a few fused DVE/ACT ops per group (hidden under the next group's
matmuls) plus tiny f32 block-ones matmuls that sum across strips.
Startup streams W1p + the first X chunks ahead of everything (X is
chunk-major in DRAM so every DMA is one contiguous block) and
pre-warms the PE clock with dummy matmuls; the symbol head + critic
(tiny) run mid-stream inside the last group; a dummy Exp activation
right after the last (table-evicting) ACT relu and a dummy Ln after the
chain's Exp preload the ACT tables off the tail's critical path.

Measured on trn2 (8 cores): ~103 us HW exec (vs 184 us for the dense
16-strip baseline), rel err ~1.5e-3 vs the fp32 reference (gate 2e-2).
"""

import os
import numpy as np

B, S, E, A = 64, 1024, 512, 128
NCORES = 8
BC = B // NCORES          # batch rows per core
H = 2 * E                 # pair-MLP hidden dim
RS = 512                  # row-slice (matmul moving free dim / PSUM bank)
KT = E // 128             # 4 k-tiles over the E features
K2 = KT // 2              # 2 fp8 DoubleRow k-tiles (256 deep)
CT = H // 128             # 8 chan tiles of the hidden dim
GMAX = 5                  # max row-slices per PSUM group (bank budget)
XW = GMAX * RS + 16       # window width (overlap col + 16B align)
CW = RS + 16              # per-strip chunk width in DRAM (513 used)

FP8_WSCALE = 32.0    # power-of-two prescale keeping fp8 W1p values mid-range
FP8_W2SCALE = 256.0  # prescale for w2p in fp8; undone exactly on chip
DESCALE = 1.0 / (FP8_WSCALE * FP8_W2SCALE)

TRACE = os.environ.get("K_TRACE", "1") == "1"

LAST_EXEC_NS = None
_CACHED = {}


def _group_sizes(nstrip):
    # balanced groups of <=GMAX strips; >=2 strips per group so PSUM
    # accumulation never serializes against its own relu
    ng = -(-nstrip // GMAX)
    base, rem = divmod(nstrip, ng)
    return [base + (1 if g < rem else 0) for g in range(ng)]


def _balance(lengths):
    """Assign 64 rows to 8 cores (8 each) minimizing max sum(lengths)."""
    lengths = np.asarray(lengths, dtype=np.int64)
    order = np.argsort(-lengths)
    bins = [[] for _ in range(NCORES)]
    sums = np.zeros(NCORES, dtype=np.int64)
    for idx in order:
        cand = [i for i in range(NCORES) if len(bins[i]) < BC]
        i = min(cand, key=lambda i: sums[i])
        bins[i].append(int(idx))
        sums[i] += lengths[idx]
    for _ in range(4000):
        hi = int(np.argmax(sums))
        improved = False
        for lo in np.argsort(sums):
            if lo == hi:
                continue
            for ai, a in enumerate(bins[hi]):
                for bi, b in enumerate(bins[lo]):
                    d = lengths[a] - lengths[b]
                    if d > 0 and max(sums[hi] - d, sums[lo] + d) < sums[hi]:
                        bins[hi][ai], bins[lo][bi] = b, a
                        sums[hi] -= d
                        sums[lo] += d
                        improved = True
                        break
                if improved:
                    break
            if improved:
                break
        if not improved:
            break
    return bins, sums


def _build(nstrip):
    import concourse.tile as tile
    from concourse import bacc, mybir

    F32 = mybir.dt.float32
    BF16 = mybir.dt.bfloat16
    F8 = mybir.dt.float8e4
    CD = BF16
    AF = mybir.ActivationFunctionType
    OP = mybir.AluOpType
    AX = mybir.AxisListType
    DR = mybir.MatmulPerfMode.DoubleRow

    gs = _group_sizes(nstrip)
    NG = len(gs)
    NP = BC * nstrip          # partitions of the packed score grids

    nc = bacc.Bacc("TRN2", target_bir_lowering=False, debug=False)

    # ---- DRAM parameters -------------------------------------------------
    # X is chunk-major: each per-strip chunk [128, 2, 513] is one fully
    # contiguous DMA read
    xt_d = nc.dram_tensor("xt8", [K2, NG, GMAX, 128, 2, CW], F8,
                          kind="ExternalInput")
    wa_d = nc.dram_tensor("wa8", [K2, 128, 2, H], F8, kind="ExternalInput")
    wb_d = nc.dram_tensor("wb8", [K2, 128, 2, H], F8, kind="ExternalInput")
    w2p_d = nc.dram_tensor("w2pj", [128, GMAX, K2 * 2, 2, 48], F8,
                           kind="ExternalInput")
    b1p_d = nc.dram_tensor("b1p_t", [128, CT], F32, kind="ExternalInput")
    seg_d = nc.dram_tensor("segadd", [NP, RS], F32, kind="ExternalInput")
    poh_d = nc.dram_tensor("paoh", [NP, RS], F32, kind="ExternalInput")
    blk_d = nc.dram_tensor("blkones", [NP, BC], F32, kind="ExternalInput")
    e12_d = nc.dram_tensor("e12t", [CT, 128, BC], CD, kind="ExternalInput")
    ws_d = nc.dram_tensor("ws", [CT, 128, H], CD, kind="ExternalInput")
    b1s_d = nc.dram_tensor("b1s_t", [128, CT], F32, kind="ExternalInput")
    w2s_d = nc.dram_tensor("w2s", [CT, 128, A], CD, kind="ExternalInput")
    b2s_d = nc.dram_tensor("b2s_row", [1, A], CD, kind="ExternalInput")
    soh_d = nc.dram_tensor("sym_onehot", [BC, A], F32, kind="ExternalInput")
    clst_d = nc.dram_tensor("clst", [KT, 128, BC], CD, kind="ExternalInput")
    wc1_d = nc.dram_tensor("wc1", [KT, 128, E], CD, kind="ExternalInput")
    bc1_d = nc.dram_tensor("bc1_t", [128, KT], F32, kind="ExternalInput")
    wc2_d = nc.dram_tensor("wc2_t", [128, KT], CD, kind="ExternalInput")
    bc2_d = nc.dram_tensor("bc2_col", [BC, 1], F32, kind="ExternalInput")
    out_d = nc.dram_tensor("out", [BC, 5], F32, kind="ExternalOutput")

    VCT = E // 128  # chan tiles of the critic hidden dim (4)

    with tile.TileContext(nc) as tc:
        with (
            tc.tile_pool(name="weights", bufs=1) as wpool,
            tc.tile_pool(name="hbuf", bufs=2) as hpool,
            tc.tile_pool(name="small", bufs=1) as spool,
            tc.tile_pool(name="psmain", bufs=1, space="PSUM") as psmain,
            tc.tile_pool(name="pssc", bufs=2, space="PSUM") as pssc,
            tc.tile_pool(name="ps3", bufs=1, space="PSUM") as ps3,
        ):
            # ---- PE clock pre-warm: dummy matmuls while the critical DMAs
            # stream, so real work starts at K=8/8 -------------------------
            warm = wpool.tile([128, RS], BF16, name="warm")
            nc.vector.memset(warm[:], 0.0)
            pw = ps3.tile([128, RS], F32, name="pw", tag="p3")
            for _ in range(6):
                nc.tensor.matmul(pw[:], warm[:, 0:128], warm[:],
                                 start=True, stop=True)

            # ---- critical DMAs first: pair-MLP weights + first X chunks --
            wa_sb = [wpool.tile([128, 2, H], F8, name=f"wa{k}") for k in range(K2)]
            wb_sb = [wpool.tile([128, 2, H], F8, name=f"wb{k}") for k in range(K2)]
            xw = {}
            for g in range(NG):
                for k2 in range(K2):
                    xw[(k2, g)] = wpool.tile(
                        [128, 2, XW], F8, name=f"x8_{k2}_{g}"
                    )

            def dma_chunk(g, k2, j):
                c0 = j * RS
                nc.sync.dma_start(
                    xw[(k2, g)][:, :, c0 : c0 + RS + 1],
                    xt_d[k2, g, j, :, :, 0 : RS + 1],
                )

            nc.sync.dma_start(wa_sb[0][:], wa_d[0, :, :, :])
            dma_chunk(0, 0, 0)
            nc.sync.dma_start(wa_sb[1][:], wa_d[1, :, :, :])
            dma_chunk(0, 1, 0)
            nc.sync.dma_start(wb_sb[0][:], wb_d[0, :, :, :])
            nc.sync.dma_start(wb_sb[1][:], wb_d[1, :, :, :])
            w2p_sb = wpool.tile([128, GMAX, K2 * 2, 2, 48], F8, name="w2p")
            nc.sync.dma_start(w2p_sb[:], w2p_d[:, :, :, :, :])
            b1p_sb = wpool.tile([128, CT], F32, name="b1p")
            nc.sync.dma_start(b1p_sb[:], b1p_d[:, :])
            for g in range(NG):
                for j in range(gs[g]):
                    if g == 0 and j == 0:
                        continue
                    for k2 in range(K2):
                        dma_chunk(g, k2, j)

            # ---- non-critical DMAs: symbol head / critic / masks ---------
            ws_sb = [wpool.tile([128, H], CD, name=f"ws{k}") for k in range(CT)]
            w2s_sb = [wpool.tile([128, A], CD, name=f"w2s{k}") for k in range(CT)]
            e12_sb = [wpool.tile([128, BC], CD, name=f"e12{k}") for k in range(CT)]
            for k in range(CT):
                nc.sync.dma_start(e12_sb[k][:], e12_d[k, :, :])
                nc.sync.dma_start(ws_sb[k][:], ws_d[k, :, :])
                nc.sync.dma_start(w2s_sb[k][:], w2s_d[k, :, :])
            b1s_sb = wpool.tile([128, CT], F32, name="b1s")
            b2s_sb = wpool.tile([1, A], CD, name="b2s")
            soh_sb = wpool.tile([BC, A], F32, name="soh")
            nc.sync.dma_start(b1s_sb[:], b1s_d[:, :])
            nc.sync.dma_start(b2s_sb[:], b2s_d[:, :])
            nc.sync.dma_start(soh_sb[:], soh_d[:, :])
            clst_sb = [wpool.tile([128, BC], CD, name=f"cls{k}") for k in range(KT)]
            wc1_sb = [wpool.tile([128, E], CD, name=f"wc1{k}") for k in range(KT)]
            for k in range(KT):
                nc.sync.dma_start(clst_sb[k][:], clst_d[k, :, :])
                nc.sync.dma_start(wc1_sb[k][:], wc1_d[k, :, :])
            bc1_sb = wpool.tile([128, KT], F32, name="bc1")
            wc2_sb = wpool.tile([128, KT], CD, name="wc2")
            bc2_sb = wpool.tile([BC, 1], F32, name="bc2")
            nc.sync.dma_start(bc1_sb[:], bc1_d[:, :])
            nc.sync.dma_start(wc2_sb[:], wc2_d[:, :])
            nc.sync.dma_start(bc2_sb[:], bc2_d[:, :])
            # per-group mask tiles (each at partition offset 0, so the
            # softmax-chain ops may span >32 partitions)
            seg_g, poh_g, blk_g = [], [], []
            p0 = 0
            for g in range(NG):
                npg = BC * gs[g]
                sg = wpool.tile([npg, RS], F32, name=f"segadd{g}")
                pg = wpool.tile([npg, RS], F32, name=f"paoh{g}")
                bg = wpool.tile([npg, BC], F32, name=f"blkones{g}")
                nc.sync.dma_start(sg[:], seg_d[p0 : p0 + npg, :])
                nc.sync.dma_start(pg[:], poh_d[p0 : p0 + npg, :])
                nc.sync.dma_start(bg[:], blk_d[p0 : p0 + npg, :])
                seg_g.append(sg)
                poh_g.append(pg)
                blk_g.append(bg)
                p0 += npg
            ones_sb = wpool.tile([1, BC], CD, name="ones")
            nc.vector.memset(ones_sb[:], 1.0)

            outbuf = spool.tile([BC, 5], F32, name="outbuf")
            nc.vector.memset(outbuf[:], 0.0)

            # per-group score grids + softmax scratch: partition 8*rs+b
            scr_g, sm_g, e_g, ps2_g, tmp_g, zs_g = [], [], [], [], [], []
            for g in range(NG):
                npg = BC * gs[g]
                scr_g.append(spool.tile([npg, RS], F32, name=f"scr{g}"))
                sm_g.append(spool.tile([npg, RS], F32, name=f"sm{g}"))
                e_g.append(spool.tile([npg, RS], F32, name=f"e{g}"))
                ps2_g.append(spool.tile([npg, RS], F32, name=f"ps2{g}"))
                tmp_g.append(spool.tile([npg, RS], F32, name=f"tmp{g}"))
                zs_g.append(spool.tile([npg, 4], F32, name=f"zs{g}"))

            smy = spool.tile([BC, A], F32, name="smy")

            def emit_symbol_critic():
                # ---- symbol head -----------------------------------------
                sh_sb = [spool.tile([128, BC], CD, name=f"sh{ct}") for ct in range(CT)]
                for ct in range(CT):
                    p3 = ps3.tile([128, BC], F32, name="p3", tag="p3")
                    for k in range(CT):
                        nc.tensor.matmul(
                            p3[:],
                            ws_sb[k][:, ct * 128 : (ct + 1) * 128],
                            e12_sb[k][:],
                            start=(k == 0),
                            stop=(k == CT - 1),
                        )
                    nc.scalar.activation(
                        sh_sb[ct][:], p3[:], AF.Relu, bias=b1s_sb[:, ct : ct + 1]
                    )
                psl = ps3.tile([BC, A], F32, name="psl", tag="p3")
                for ct in range(CT):
                    nc.tensor.matmul(
                        psl[:], sh_sb[ct][:], w2s_sb[ct][:], start=(ct == 0), stop=False
                    )
                nc.tensor.matmul(
                    psl[:], ones_sb[:], b2s_sb[:], start=False, stop=True
                )
                nc.vector.tensor_copy(smy[:], psl[:])
                emit_symbol_softmax()

                # ---- critic ----------------------------------------------
                hc_sb = [spool.tile([128, BC], CD, name=f"hc{ct}") for ct in range(VCT)]
                for ct in range(VCT):
                    pc = ps3.tile([128, BC], F32, name="pc", tag="p3")
                    for k in range(KT):
                        nc.tensor.matmul(
                            pc[:],
                            wc1_sb[k][:, ct * 128 : (ct + 1) * 128],
                            clst_sb[k][:],
                            start=(k == 0),
                            stop=(k == KT - 1),
                        )
                    nc.scalar.activation(
                        hc_sb[ct][:], pc[:], AF.Relu, bias=bc1_sb[:, ct : ct + 1]
                    )
                pv = ps3.tile([BC, 1], F32, name="pv", tag="p3")
                for ct in range(VCT):
                    nc.tensor.matmul(
                        pv[:], hc_sb[ct][:], wc2_sb[:, ct : ct + 1],
                        start=(ct == 0), stop=(ct == VCT - 1),
                    )
                nc.vector.tensor_add(outbuf[:, 2:3], pv[:], bc2_sb[:])  # val

            def emit_symbol_softmax():
                mny = spool.tile([BC, 1], F32, name="mny")
                nc.vector.tensor_reduce(mny[:], smy[:], axis=AX.X, op=OP.max, negate=True)
                pey = spool.tile([BC, A], F32, name="pey")
                zsy = spool.tile([BC, 1], F32, name="zsy")
                nc.scalar.activation(
                    pey[:], smy[:], AF.Exp, bias=mny[:, 0:1], accum_out=zsy[:]
                )
                p2y = spool.tile([BC, A], F32, name="p2y")
                s2y = spool.tile([BC, 1], F32, name="s2y")
                nc.vector.scalar_tensor_tensor(
                    p2y[:], pey[:], 1.0, smy[:], OP.mult, OP.mult, accum_out=s2y[:]
                )
                lzy = spool.tile([BC, 1], F32, name="lzy")
                nc.scalar.activation(lzy[:], zsy[:], AF.Ln)
                lsey = spool.tile([BC, 1], F32, name="lsey")
                nc.vector.tensor_sub(lsey[:], lzy[:], mny[:])
                tmpy = spool.tile([BC, A], F32, name="tmpy")
                say = spool.tile([BC, 1], F32, name="say")
                nc.vector.scalar_tensor_tensor(
                    tmpy[:], smy[:], 1.0, soh_sb[:], OP.mult, OP.mult, accum_out=say[:]
                )
                rzy = spool.tile([BC, 1], F32, name="rzy")
                nc.vector.reciprocal(rzy[:], zsy[:])
                s2zy = spool.tile([BC, 1], F32, name="s2zy")
                nc.vector.tensor_mul(s2zy[:], s2y[:], rzy[:])
                nc.vector.tensor_sub(outbuf[:, 1:2], say[:], lsey[:])   # logp_sym
                nc.vector.tensor_sub(outbuf[:, 4:5], lsey[:], s2zy[:])  # ent_sym

            def emit_chain(g, preload_ln=False):
                """Masked softmax partials for group g's strips: per-partition
                (= per (strip, b)) sums; cross-strip combine happens later in
                the block-ones matmuls.  The spa product reads the raw score
                grid so it fills the DVE while ACT runs the Exp."""
                nc.vector.tensor_add(sm_g[g][:], scr_g[g][:], seg_g[g][:])
                nc.vector.scalar_tensor_tensor(
                    tmp_g[g][:], scr_g[g][:], 1.0, poh_g[g][:],
                    OP.mult, OP.mult, accum_out=zs_g[g][:, 2:3],
                )
                nc.scalar.activation(
                    e_g[g][:], sm_g[g][:], AF.Exp, accum_out=zs_g[g][:, 0:1]
                )
                if preload_ln:
                    # pull the Ln table load off the critical path: it runs
                    # on ACT while the DVE finishes the reduction below
                    dum2 = spool.tile([1, 1], F32, name="dum2")
                    nc.scalar.activation(dum2[:], b1p_sb[0:1, 0:1], AF.Ln)
                nc.vector.scalar_tensor_tensor(
                    ps2_g[g][:], e_g[g][:], 1.0, sm_g[g][:],
                    OP.mult, OP.mult, accum_out=zs_g[g][:, 1:2],
                )

            # ---- main pair-MLP over groups of <=GMAX row slices ----------
            soff = 0
            for g in range(NG):
                qsg = gs[g]
                ps_q = [
                    psmain.tile([128, RS], F32, name=f"ps{j}", tag=f"ps{j}")
                    for j in range(qsg)
                ]
                hs = {}
                for ct in range(CT):
                    # group 0 / ct 0 runs j-outer so the PE consumes the
                    # arriving X chunks in stream order at kernel start;
                    # the last ct runs j-outer so each slice's final psum
                    # closes early and the relu -> score-matmul epilogue
                    # pipelines without a PE bubble
                    wj = (
                        [(w, j) for j in range(qsg) for w in range(2 * K2)]
                        if (g == 0 and ct == 0) or ct == CT - 1
                        else [(w, j) for w in range(2 * K2) for j in range(qsg)]
                    )
                    for w, j in wj:
                        ab, k2 = divmod(w, K2)
                        wsb = (wa_sb if ab == 0 else wb_sb)[k2]
                        nc.tensor.matmul(
                            ps_q[j][:],
                            wsb[:, :, ct * 128 : (ct + 1) * 128],
                            xw[(k2, g)][:, :, j * RS + ab : j * RS + ab + RS],
                            start=(w == 0),
                            stop=(w == 2 * K2 - 1),
                            perf_mode=DR,
                        )
                    m, jj = divmod(ct, 2)
                    for j in range(qsg):
                        key = (m, j)
                        if key not in hs:
                            hs[key] = hpool.tile(
                                [128, 2, RS], F8, name=f"h8_{m}_{j}",
                                tag=f"h8_{m}_{j}",
                            )
                        plane = hs[key][:, jj, :]
                        # split bias+relu ~3:1 DVE:ACT (ACT's fp8 path is
                        # ~2.4x slower; both stay under the PE shadow)
                        if (ct * qsg + j) % 3 == 2:
                            nc.scalar.activation(
                                plane, ps_q[j][:], AF.Relu,
                                bias=b1p_sb[:, ct : ct + 1],
                            )
                        else:
                            nc.vector.tensor_scalar(
                                plane, ps_q[j][:],
                                b1p_sb[:, ct : ct + 1], 0.0,
                                OP.add, OP.max,
                            )
                    # slot the (tiny) symbol head + critic mid-stream: their
                    # DMAs have long landed and their DVE/ACT chain hides
                    # under this group's remaining matmuls
                    if g == min(1, NG - 1) and ct == 3:
                        emit_symbol_critic()
                    # ACT relus evict the Exp table; reload it right after
                    # the last relu so the load hides under the score matmuls
                    if g == NG - 1 and ct == CT - 1:
                        dum = spool.tile([1, 1], F32, name="dum")
                        nc.scalar.activation(dum[:], b1p_sb[0:1, 0:1], AF.Exp)
                # all strips of the group accumulate into one M=8*qsg
                # score PSUM: strip j's stationary is nonzero only at
                # columns [8j, 8j+8), so the accumulation adds zeros
                # elsewhere; one ACT copy then descales the whole group
                # straight into the score grid (no DMA, no sem latency)
                npg = BC * qsg
                psG = pssc.tile([npg, RS], F32, name="psG", tag="psd")
                n = 0
                for j in range(qsg):
                    for m in range(CT // 2):
                        nc.tensor.matmul(
                            psG[:],
                            w2p_sb[:, j, m, :, 0:npg],
                            hs[(m, j)][:, :, :],
                            start=(n == 0),
                            stop=(n == 4 * qsg - 1),
                            perf_mode=DR,
                        )
                        n += 1
                nc.scalar.activation(
                    scr_g[g][:], psG[:], AF.Copy, bias=0.0, scale=DESCALE
                )
                soff += qsg
                # group g's strips are all landed; its chain runs on DVE/ACT
                # under group g+1's matmuls (only the last group's chain is
                # partially exposed in the tail)
                emit_chain(g, preload_ln=(g == NG - 1))

            # ---- combine strips: tiny f32 matmuls + pointwise tail -------
            pz = ps3.tile([BC, 4], F32, name="pz", tag="p3")
            for g in range(NG):
                nc.tensor.matmul(pz[:], blk_g[g][:], zs_g[g][:],
                                 start=(g == 0), stop=(g == NG - 1))
            # final pointwise ops read the PSUM accumulators directly
            lse = spool.tile([BC, 1], F32, name="lse")
            nc.scalar.activation(lse[:], pz[:, 0:1], AF.Ln)
            rz = spool.tile([BC, 1], F32, name="rz")
            nc.vector.reciprocal(rz[:], pz[:, 0:1])
            s2z = spool.tile([BC, 1], F32, name="s2z")
            nc.vector.tensor_mul(s2z[:], pz[:, 1:2], rz[:])
            nc.vector.tensor_sub(outbuf[:, 0:1], pz[:, 2:3], lse[:])  # logp_pos
            nc.vector.tensor_sub(outbuf[:, 3:4], lse[:], s2z[:])      # ent_pos

            nc.sync.dma_start(out_d[:, :], outbuf[:])

    nc.compile()
    return nc


def _to_cd(arr):
    import ml_dtypes

    return np.ascontiguousarray(arr).astype(ml_dtypes.bfloat16)


def _to_f8(arr):
    import ml_dtypes

    return np.ascontiguousarray(arr).astype(ml_dtypes.float8_e4m3)


def _ensure_axon_hooks():
    """bass_utils imports antenv.axon_hooks unconditionally when tracing
    under axon; provide a registry if the image lacks it, and register the
    ctypes NTFF hook that trn_boot would have installed had the module
    existed at boot time."""
    try:
        import antenv.axon_hooks  # noqa: F401
        return
    except ImportError:
        pass
    import sys
    import types

    try:
        import antenv
    except ImportError:
        return
    mod = types.ModuleType("antenv.axon_hooks")
    mod._hook = None
    mod.set_axon_ntff_profile_hook = lambda h: setattr(mod, "_hook", h)
    mod.get_axon_ntff_profile_hook = lambda: mod._hook
    sys.modules["antenv.axon_hooks"] = mod
    antenv.axon_hooks = mod
    try:
        from trn_agent_boot.trn_boot import _ntff_profile_via_ctypes

        so_path = "/opt/axon/libaxon_pjrt.so"
        if os.path.exists(so_path):
            mod.set_axon_ntff_profile_hook(_ntff_profile_via_ctypes(so_path))
    except Exception:
        pass


def kernel(**inputs):
    global LAST_EXEC_NS
    import ml_dtypes
    from concourse.bass_utils import run_bass_kernel_spmd

    _ensure_axon_hooks()

    f32 = np.float32
    states = np.asarray(inputs["states"], f32)
    cls_token = np.asarray(inputs["cls_token"], f32)
    W1p = np.asarray(inputs["W1p"], f32)
    b1p = np.asarray(inputs["b1p"], f32)
    w2p = np.asarray(inputs["w2p"], f32)
    W1s = np.asarray(inputs["W1s"], f32)
    b1s = np.asarray(inputs["b1s"], f32)
    W2s = np.asarray(inputs["W2s"], f32)
    b2s = np.asarray(inputs["b2s"], f32)
    Wc1 = np.asarray(inputs["Wc1"], f32)
    bc1 = np.asarray(inputs["bc1"], f32)
    wc2 = np.asarray(inputs["wc2"], f32)
    bc2 = np.asarray(inputs["bc2"], f32)
    lengths = np.asarray(inputs["lengths"]).astype(np.int64)
    position_action = np.asarray(inputs["position_action"]).astype(np.int64)
    symbol_action = np.asarray(inputs["symbol_action"]).astype(np.int64)

    bins, sums = _balance(lengths)
    nstrip = max(1, int(-(-int(sums.max()) // RS)))
    if "K_NSTRIP" in os.environ:
        nstrip = max(nstrip, int(os.environ["K_NSTRIP"]))
    cap = nstrip * RS
    gs = _group_sizes(nstrip)
    NG = len(gs)
    NP = BC * nstrip

    # ---- shared (replicated) weight prep --------------------------------
    # DoubleRow layout: [k2, p, j, m] = W[256*k2 + 128*j + p, m] * scale
    wa4 = W1p[:E].reshape(K2, 2, 128, H).transpose(0, 2, 1, 3)
    wb4 = W1p[E:].reshape(K2, 2, 128, H).transpose(0, 2, 1, 3)
    shared = {
        "wa8": _to_f8(wa4 * FP8_WSCALE),
        "wb8": _to_f8(wb4 * FP8_WSCALE),
        "b1p_t": np.ascontiguousarray(b1p.reshape(CT, 128).T * FP8_WSCALE, dtype=f32),
        "ws": _to_cd(W1s.reshape(CT, 128, H)),
        "b1s_t": np.ascontiguousarray(b1s.reshape(CT, 128).T, dtype=f32),
        "w2s": _to_cd(W2s.reshape(CT, 128, A)),
        "b2s_row": _to_cd(b2s.reshape(1, A)),
        "wc1": _to_cd(Wc1.reshape(KT, 128, E)),
        "bc1_t": np.ascontiguousarray(bc1.reshape(KT, 128).T, dtype=f32),
        "wc2_t": _to_cd(wc2.reshape(KT, 128).T),
        "bc2_col": np.full((BC, 1), bc2[0], dtype=f32),
    }
    # w2p, replicated into per-strip column blocks: strip j's stationary
    # occupies columns [8j, 8j+8) of the group-wide M=8*qsg score matmul
    w2pdr = w2p.reshape(K2 * 2, 2, 128).transpose(2, 0, 1)  # [p, m, pl]
    w2pj = np.zeros((128, GMAX, K2 * 2, 2, 48), f32)
    for j in range(GMAX):
        w2pj[:, j, :, :, BC * j : BC * (j + 1)] = w2pdr[..., None]
    shared["w2pj"] = _to_f8(w2pj * FP8_W2SCALE)
    # block-ones: partition p = 8*rs + b sums into output row b
    blk = np.zeros((NP, BC), f32)
    blk[np.arange(NP), np.arange(NP) % BC] = 1.0
    shared["blkones"] = blk

    in_maps = []
    bidx = np.arange(BC)
    for c in range(NCORES):
        rows = bins[c]
        ln = lengths[rows]
        offs = np.zeros(BC + 1, np.int64)
        offs[1:] = np.cumsum(ln)
        V = int(offs[-1])

        # pack columns: states[row, 0:len] contiguous; +1 overlap col padded
        xp = np.zeros((cap + 1, E), ml_dtypes.float8_e4m3)
        for b, r in enumerate(rows):
            xp[offs[b] : offs[b + 1]] = states[r, : ln[b]].astype(
                ml_dtypes.float8_e4m3
            )
        x8 = xp.T  # [E, cap+1]
        xt8 = np.zeros((K2, NG, GMAX, 128, 2, CW), ml_dtypes.float8_e4m3)
        goff = 0
        for g in range(NG):
            for j in range(gs[g]):
                c0 = goff + j * RS
                for k2 in range(K2):
                    for pl in range(2):
                        xt8[k2, g, j, :, pl, : RS + 1] = x8[
                            256 * k2 + 128 * pl : 256 * k2 + 128 * (pl + 1),
                            c0 : c0 + RS + 1,
                        ]
            goff += gs[g] * RS

        # segment masks on the packed grid: partition 8*rs + b
        pa = position_action[rows]
        sa = symbol_action[rows]
        vm = np.zeros((BC, cap), bool)
        poh = np.zeros((BC, cap), f32)
        for b in range(BC):
            vm[b, offs[b] : offs[b + 1] - 1] = True  # pairs t < len-1
            poh[b, offs[b] + pa[b]] = 1.0
        segadd = np.where(vm, 0.0, -1e30).astype(f32)
        segadd = segadd.reshape(BC, nstrip, RS).transpose(1, 0, 2).reshape(NP, RS)
        poh = poh.reshape(BC, nstrip, RS).transpose(1, 0, 2).reshape(NP, RS)

        sym_onehot = np.zeros((BC, A), f32)
        sym_onehot[bidx, sa] = 1.0
        st = states[rows]  # (BC, S, E)
        e12 = np.concatenate([st[bidx, pa], st[bidx, pa + 1]], axis=1)  # (BC, 2E)

        m = dict(shared)
        m["xt8"] = xt8
        m["segadd"] = np.ascontiguousarray(segadd)
        m["paoh"] = np.ascontiguousarray(poh)
        m["sym_onehot"] = sym_onehot
        m["e12t"] = _to_cd(e12.T.reshape(CT, 128, BC))
        m["clst"] = _to_cd(cls_token[rows].T.reshape(KT, 128, BC))
        in_maps.append(m)

    if nstrip not in _CACHED:
        _CACHED[nstrip] = _build(nstrip)
    nc = _CACHED[nstrip]

    # cold first execution of a freshly-loaded NEFF measures ~15-20% slow
    # (device-side warmup); run once untimed, then the traced run
    run_bass_kernel_spmd(nc, in_maps, core_ids=list(range(NCORES)), trace=False)
    try:
        res = run_bass_kernel_spmd(
            nc, in_maps, core_ids=list(range(NCORES)), trace=TRACE
        )
    except (ImportError, ModuleNotFoundError):
        res = run_bass_kernel_spmd(
            nc, in_maps, core_ids=list(range(NCORES)), trace=False
        )
    LAST_EXEC_NS = res.exec_time_ns

    full = np.zeros((B, 5), f32)
    for c in range(NCORES):
        full[bins[c]] = np.asarray(res.results[c]["out"])
    return np.ascontiguousarray(full.T, dtype=f32)  # (5, 64)
